# revision 1
# baseline (speedup 1.0000x reference)
"""Trainium2 Bass kernel for nn_AtNeuron_18622978195626.

Temporal diff-coding scan over T=8 steps of batched 512x512x512 matmuls:
    inputs x, y: [(T+1)*B, 512, 512] = [9, 8, 512, 512], out[0] = 0
    step t=1..8:  yv += y_t / t
                  out_t = x_t @ yv + xv @ y_t      (xv = carry before update)
                  xv += x_t / t
(The reference's 3-matmul form  x_t@y_t/t + x_t@yv_old + xv@y_t  folds into
2 matmuls because  x_t@y_t/t + x_t@yv_old = x_t@(yv_old + y_t/t).)

Sharding: batch dim B=8, one batch element per NeuronCore (data parallel, no
communication). x is transposed on the host during sharding so it lands in
SBUF partition-on-k ([K, M]) as the PE's stationary operand requires; y's
natural layout [K, N] already suits the moving operand. Matmuls run in
float32r (full-rate fp32 path, ~1e-3 rel err, well inside the 2e-2 gate).
"""

import sys

if "/opt/trn_rl_repo" not in sys.path:
    sys.path.insert(0, "/opt/trn_rl_repo")

import numpy as np

import concourse.mybir as mybir
import concourse.tile as tile
from concourse import bacc
from concourse.bass_utils import run_bass_kernel_spmd

T = 8          # scan steps (t = 1..8); t=0 output is identically zero
B = 8          # batch = number of cores
D = 512        # matrix dim
P = 128        # partitions
KO = D // P    # k/m outer tiles = 4

MM_DT = mybir.dt.float32r   # full-rate fp32 matmul path
F32 = mybir.dt.float32

_CACHE = {}


def _build():
    """Build + compile the single-core program (same program on all 8 cores)."""
    if "nc" in _CACHE:
        return _CACHE["nc"]

    nc = bacc.Bacc("TRN2", target_bir_lowering=False, debug=False)
    # xT[t] is x_{t+1}.T, layout [K, M]; y[t] is y_{t+1}, layout [K, N]
    xT_d = nc.dram_tensor("xT", [T, D, D], MM_DT, kind="ExternalInput").ap()
    y_d = nc.dram_tensor("y", [T, D, D], MM_DT, kind="ExternalInput").ap()
    o_d = nc.dram_tensor("out", [T, D, D], F32, kind="ExternalOutput").ap()

    with tile.TileContext(nc) as tc:
        with (
            tc.tile_pool(name="xin", bufs=T) as xpool,
            tc.tile_pool(name="yin", bufs=T) as ypool,
            tc.tile_pool(name="yvp", bufs=2) as yvpool,
            tc.tile_pool(name="xvp", bufs=2) as xvpool,
            tc.tile_pool(name="outs", bufs=2) as opool,
            tc.tile_pool(name="psum", bufs=8, space="PSUM") as pspool,
        ):
            # Load everything up front. nc.sync's HWDGE ring is FIFO and each
            # dma_start internally fans out over all 16 SDMA engines, so loads
            # complete in step order at full bandwidth.
            xts, yts = [], []
            for t in range(T):
                xt = xpool.tile([P, KO, D], MM_DT, tag="xT")
                nc.sync.dma_start(
                    xt[:], xT_d[t].rearrange("(ko ki) m -> ki ko m", ki=P)
                )
                xts.append(xt)
                yt = ypool.tile([P, KO, D], MM_DT, tag="y")
                nc.sync.dma_start(
                    yt[:], y_d[t].rearrange("(ko ki) n -> ki ko n", ki=P)
                )
                yts.append(yt)

            yv = yts[0]   # yv after step 1 == y_1 (inv = 1)
            xvT = xts[0]  # xv after step 1 == x_1
            for s in range(T):
                inv = 1.0 / (s + 1)
                if s > 0:
                    # carry for this step, into a fresh tile so the update
                    # pipelines ahead of the PE instead of serializing on WAR
                    yv_new = yvpool.tile([P, KO, D], MM_DT, tag="yv")
                    nc.vector.scalar_tensor_tensor(
                        yv_new[:], yts[s][:], inv, yv[:],
                        mybir.AluOpType.mult, mybir.AluOpType.add,
                    )
                    yv = yv_new

                out_s = opool.tile([P, KO, D], F32, tag="out")
                n_mm = KO if s == 0 else 2 * KO
                for mo in range(KO):
                    ps = pspool.tile([P, D], F32, tag="ps")
                    i = 0
                    for k in range(KO):  # out1 = x_t @ yv
                        nc.tensor.matmul(
                            ps[:], xts[s][:, k, mo * P:(mo + 1) * P], yv[:, k, :],
                            start=(i == 0), stop=(i == n_mm - 1),
                        )
                        i += 1
                    if s > 0:            # out2 = xv_old @ y_t
                        for k in range(KO):
                            nc.tensor.matmul(
                                ps[:], xvT[:, k, mo * P:(mo + 1) * P], yts[s][:, k, :],
                                start=False, stop=(i == n_mm - 1),
                            )
                            i += 1
                    # drain PSUM quickly; split across DVE and ACT
                    if mo % 2 == 0:
                        nc.vector.tensor_copy(out_s[:, mo, :], ps[:])
                    else:
                        nc.scalar.copy(out_s[:, mo, :], ps[:])

                # stores ride ACT's separate HWDGE ring, off the load FIFO
                nc.scalar.dma_start(
                    o_d[s].rearrange("(mo mi) n -> mi mo n", mi=P), out_s[:]
                )

                if 0 < s < T - 1:  # xv carry for the next step
                    xv_new = xvpool.tile([P, KO, D], MM_DT, tag="xvT")
                    nc.vector.scalar_tensor_tensor(
                        xv_new[:], xts[s][:], inv, xvT[:],
                        mybir.AluOpType.mult, mybir.AluOpType.add,
                    )
                    xvT = xv_new
                elif s == 0:
                    xvT = xts[0]

    nc.compile()
    _CACHE["nc"] = nc
    return nc


def _run(inputs, trace=False):
    x = np.ascontiguousarray(np.asarray(inputs["x"], dtype=np.float32))
    y = np.ascontiguousarray(np.asarray(inputs["y"], dtype=np.float32))
    x5 = x.reshape(T + 1, B, D, D)
    y5 = y.reshape(T + 1, B, D, D)

    in_maps = []
    for c in range(B):
        in_maps.append({
            "xT": np.ascontiguousarray(x5[1:, c].transpose(0, 2, 1)),
            "y": np.ascontiguousarray(y5[1:, c]),
        })

    nc = _build()
    res = run_bass_kernel_spmd(nc, in_maps, core_ids=list(range(B)), trace=trace)

    out = np.zeros((T + 1, B, D, D), dtype=np.float32)
    for c in range(B):
        out[1:, c] = res.results[c]["out"]
    return out.reshape((T + 1) * B, D, D), res


def kernel(**inputs) -> np.ndarray:
    out, _ = _run(inputs, trace=False)
    return out


def kernel_traced(inputs):
    """Like kernel() but with NTFF profiling; returns (out, BassKernelResults)."""
    return _run(inputs, trace=True)


# revision 2
# speedup vs baseline: 1.0346x; 1.0346x over previous
"""Trainium2 Bass kernel for nn_AtNeuron_18622978195626.

Temporal diff-coding scan over T=8 steps of batched 512x512x512 matmuls:
    inputs x, y: [(T+1)*B, 512, 512] = [9, 8, 512, 512], out[0] = 0
    step t=1..8:  yv += y_t / t
                  out_t = x_t @ yv + xv @ y_t      (xv = carry before update)
                  xv += x_t / t
(The reference's 3-matmul form  x_t@y_t/t + x_t@yv_old + xv@y_t  folds into
2 matmuls because  x_t@y_t/t + x_t@yv_old = x_t@(yv_old + y_t/t).)

Sharding: batch dim B=8, one batch element per NeuronCore (data parallel, no
communication). x is transposed on the host during sharding so it lands in
SBUF partition-on-k ([K, M]) as the PE's stationary operand requires; y's
natural layout [K, N] already suits the moving operand. Matmuls run in
float32r (full-rate fp32 path, ~2e-4 rel err, well inside the 2e-2 gate).

Everything (loads, carry updates, PSUM drains, stores) is chunked at
k-chunk granularity ([128, 512] = 256 KB) so the PE can start each
accumulation group as soon as its chunk arrives rather than stalling on
full-matrix DMAs.
"""

import sys

if "/opt/trn_rl_repo" not in sys.path:
    sys.path.insert(0, "/opt/trn_rl_repo")

import numpy as np

import concourse.mybir as mybir
import concourse.tile as tile
from concourse import bacc
from concourse.bass_utils import run_bass_kernel_spmd

T = 8          # scan steps (t = 1..8); t=0 output is identically zero
B = 8          # batch = number of cores
D = 512        # matrix dim
P = 128        # partitions
KO = D // P    # k/m outer tiles = 4

MM_DT = mybir.dt.float32r   # full-rate fp32 matmul path
F32 = mybir.dt.float32

_CACHE = {}


def _build():
    """Build + compile the single-core program (same program on all 8 cores)."""
    if "nc" in _CACHE:
        return _CACHE["nc"]

    nc = bacc.Bacc("TRN2", target_bir_lowering=False, debug=False)
    # xT[t] is x_{t+1}.T, layout [K, M]; y[t] is y_{t+1}, layout [K, N]
    xT_d = nc.dram_tensor("xT", [T, D, D], MM_DT, kind="ExternalInput").ap()
    y_d = nc.dram_tensor("y", [T, D, D], MM_DT, kind="ExternalInput").ap()
    o_d = nc.dram_tensor("out", [T, D, D], F32, kind="ExternalOutput").ap()

    with tile.TileContext(nc) as tc:
        with (
            tc.tile_pool(name="xin", bufs=T * KO) as xpool,
            tc.tile_pool(name="yin", bufs=T * KO) as ypool,
            tc.tile_pool(name="yvp", bufs=2 * KO) as yvpool,
            tc.tile_pool(name="xvp", bufs=2 * KO) as xvpool,
            tc.tile_pool(name="outs", bufs=4) as opool,
            tc.tile_pool(name="psum", bufs=2, space="PSUM") as pspool,
        ):
            # Chunked loads in step order on nc.sync's FIFO HWDGE ring; each
            # chunk is a contiguous 256 KB block of DRAM.
            xch = [[None] * KO for _ in range(T)]
            ych = [[None] * KO for _ in range(T)]
            for t in range(T):
                for k in range(KO):
                    xc = xpool.tile([P, D], MM_DT, tag="xT")
                    nc.sync.dma_start(xc[:], xT_d[t, k * P:(k + 1) * P, :])
                    xch[t][k] = xc
                    yc = ypool.tile([P, D], MM_DT, tag="y")
                    nc.sync.dma_start(yc[:], y_d[t, k * P:(k + 1) * P, :])
                    ych[t][k] = yc

            yv = ych[0]   # yv after step 1 == y_1 (inv = 1)
            xvT = xch[0]  # xv after step 1 == x_1
            for s in range(T):
                inv = 1.0 / (s + 1)
                if s > 0:
                    # per-chunk carry update into fresh tiles (pipelines ahead)
                    yv_new = []
                    for k in range(KO):
                        c = yvpool.tile([P, D], MM_DT, tag="yv")
                        nc.vector.scalar_tensor_tensor(
                            c[:], ych[s][k][:], inv, yv[k][:],
                            mybir.AluOpType.mult, mybir.AluOpType.add,
                        )
                        yv_new.append(c)
                    yv = yv_new

                ps = pspool.tile([P, KO, D], F32, tag="ps")
                n_mm = KO if s == 0 else 2 * KO
                for mo in range(KO):
                    i = 0
                    for k in range(KO):  # out1 = x_t @ yv
                        nc.tensor.matmul(
                            ps[:, mo, :], xch[s][k][:, mo * P:(mo + 1) * P], yv[k][:],
                            start=(i == 0), stop=(i == n_mm - 1),
                        )
                        i += 1
                    if s > 0:            # out2 = xv_old @ y_t
                        for k in range(KO):
                            nc.tensor.matmul(
                                ps[:, mo, :], xvT[k][:, mo * P:(mo + 1) * P], ych[s][k][:],
                                start=False, stop=(i == n_mm - 1),
                            )
                            i += 1

                # drain PSUM in halves on ACT; stores ride ACT's HWDGE ring
                for h in range(2):
                    oh = opool.tile([P, 2, D], F32, tag="out")
                    nc.scalar.copy(oh[:], ps[:, 2 * h:2 * h + 2, :])
                    nc.scalar.dma_start(
                        o_d[s, 2 * h * P:(2 * h + 2) * P, :].rearrange(
                            "(mo mi) n -> mi mo n", mi=P),
                        oh[:],
                    )

                if 0 < s < T - 1:  # xv carry for the next step
                    xv_new = []
                    for k in range(KO):
                        c = xvpool.tile([P, D], MM_DT, tag="xvT")
                        nc.vector.scalar_tensor_tensor(
                            c[:], xch[s][k][:], inv, xvT[k][:],
                            mybir.AluOpType.mult, mybir.AluOpType.add,
                        )
                        xv_new.append(c)
                    xvT = xv_new

    nc.compile()
    _CACHE["nc"] = nc
    return nc


def _run(inputs, trace=False):
    x = np.ascontiguousarray(np.asarray(inputs["x"], dtype=np.float32))
    y = np.ascontiguousarray(np.asarray(inputs["y"], dtype=np.float32))
    x5 = x.reshape(T + 1, B, D, D)
    y5 = y.reshape(T + 1, B, D, D)

    in_maps = []
    for c in range(B):
        in_maps.append({
            "xT": np.ascontiguousarray(x5[1:, c].transpose(0, 2, 1)),
            "y": np.ascontiguousarray(y5[1:, c]),
        })

    nc = _build()
    res = run_bass_kernel_spmd(nc, in_maps, core_ids=list(range(B)), trace=trace)

    out = np.zeros((T + 1, B, D, D), dtype=np.float32)
    for c in range(B):
        out[1:, c] = res.results[c]["out"]
    return out.reshape((T + 1) * B, D, D), res


def kernel(**inputs) -> np.ndarray:
    out, _ = _run(inputs, trace=False)
    return out


def kernel_traced(inputs):
    """Like kernel() but with NTFF profiling; returns (out, BassKernelResults)."""
    return _run(inputs, trace=True)
